# revision 1
# baseline (speedup 1.0000x reference)
"""CYK/PCFG inside-algorithm kernel for Trainium2 (8 NeuronCores).

Problem: R=96 nonterminals, 96 sentences x 24 tokens.
  rules = softmax(binary_logits over (y,z)); start = softmax(start_logits)
  chart DP over span length; out[b] = start . chart[b, 0, n-1]

Sharding: data-parallel over sentences, 12 per core; rules replicated.

Device layout (per core):
  - 12 sentences split into G=4 partition-groups x SB=3 sentences
    (sentence b = 3*g + b_l).
  - L stack:  L[32g + k, (b_l, p, y)]  = chart[b, p, p+k]        (left ops)
  - RB stack: RB[32g + k, (b_l, p, z)] = chart[b, p+k+1, p+s-1]  (right ops,
      ping-pong; shifted down one partition row per span via one DMA)
  - pair matmul (per item): out[z,y] = sum_k RB[k,z] * L[k,y], K=s-1<=23;
    four concurrent row-group matmuls at partition bases 0/32/64/96.
  - val matmul: out[x, items] accumulated over y=0..95 with
    lhsT = rulesYX[:, y*96:+96] ([z,x]) and rhs = pairT (stride-96 items).
  - val results PE-transposed and DMA-scattered back into L and RB.

Numerics: terminal init = SCALE(=96) so chart values ~ Catalan numbers,
keeping fp32 comfortably in range (true outputs ~1e-37). Host divides by
SCALE**n in float64 at the end.
"""

import math
import sys
from contextlib import ExitStack

import numpy as np

_REPO = "/opt/trn_rl_repo"
if _REPO not in sys.path:
    sys.path.insert(0, _REPO)

import concourse.bass as bass  # noqa: E402,F401
import concourse.tile as tile  # noqa: E402
from concourse import bacc, mybir  # noqa: E402
from concourse.bass_utils import run_bass_kernel_spmd  # noqa: E402
from concourse.masks import make_identity  # noqa: E402

R = 96          # nonterminals
NTOK = 24       # sentence length
NCORES = 8
BLOC = 12       # sentences per core
G = 4           # partition groups
SB = 3          # sentences per group
SCALE = 96.0
WCAP = 96       # pairT half capacity (item slots)

F32 = mybir.dt.float32
BF16 = mybir.dt.bfloat16

# --- precision mode ----------------------------------------------------------
# "f32": full fp32 (2-pass matmuls, slow weight loads) ~4e-8 rel err
# "bf16": bf16 operands, 128-padded stationary tiles (FWL) ~3e-3 rel err
import os as _os  # noqa: E402

MODE = _os.environ.get("KERNEL_MODE", "f32")
if MODE == "bf16":
    CHART_DT = BF16   # L/RB stacks (pair-matmul operands)
    PAIRT_DT = BF16   # pair staging in SBUF (val-matmul rhs)
    RULES_DT = BF16   # rulesYX (val-matmul lhsT)
    ZPAD = 128        # RB slot width (pair lhsT columns; 128 => FWL)
    XPAD = 128        # rules slot width (val lhsT columns; 128 => FWL)
else:
    CHART_DT = F32
    PAIRT_DT = F32
    RULES_DT = F32
    ZPAD = 96
    XPAD = 96
VAL_F32R = False  # run val matmuls in float32r mode (bitcast)
SEQUENTIAL = False  # debug: emit strictly sequentially (no pipelining)
PAIR_F32R = False


def _windows(n_l: int, cap: int = WCAP // G) -> list[tuple[int, int]]:
    """Split n_l l-indices (l = b_l*P + p) into ~even windows <= cap."""
    nw = math.ceil(n_l / cap)
    base = math.ceil(n_l / nw)
    out = []
    i0 = 0
    while i0 < n_l:
        i1 = min(i0 + base, n_l)
        out.append((i0, i1))
        i0 = i1
    return out


def build_program(n: int = NTOK):
    """Build the SPMD Bass program for one core (n tokens per sentence)."""
    nc = bacc.Bacc(
        "TRN2",
        target_bir_lowering=False,
        debug=False,
        enable_asserts=False,
        num_devices=NCORES,
    )

    d_logits = nc.dram_tensor("blogits", [R, R * R], F32, kind="ExternalInput").ap()
    d_slog = nc.dram_tensor("slog", [1, R], F32, kind="ExternalInput").ap()
    d_oh = nc.dram_tensor("oh", [BLOC, n, R], CHART_DT, kind="ExternalInput").ap()
    d_out = nc.dram_tensor("out", [1, BLOC], F32, kind="ExternalOutput").ap()

    with tile.TileContext(nc) as tc, ExitStack() as ctx:
        p_persist = ctx.enter_context(tc.tile_pool(name="persist", bufs=1))
        # "big" pool: softmax workspace early, then the two pairT halves
        p_big = ctx.enter_context(tc.tile_pool(name="big", bufs=2))
        p_small = ctx.enter_context(tc.tile_pool(name="small", bufs=4))
        p_valsb = ctx.enter_context(tc.tile_pool(name="valsb", bufs=2))
        p_valt = ctx.enter_context(tc.tile_pool(name="valt", bufs=4))
        pp_pair = ctx.enter_context(tc.tile_pool(name="ppair", bufs=4, space="PSUM"))
        pp_val = ctx.enter_context(tc.tile_pool(name="pval", bufs=2, space="PSUM"))
        pp_tr = ctx.enter_context(tc.tile_pool(name="ptr", bufs=2, space="PSUM"))

        # ---- persistent tiles (distinct tags: each needs its own slot) ----
        rulesYX = p_persist.tile([R, R * XPAD], RULES_DT, tag="rules")
        L = p_persist.tile([128, SB * n * R], CHART_DT, tag="L")
        RBa = p_persist.tile([128, SB * n * ZPAD], CHART_DT, tag="RBa")
        RBb = p_persist.tile([128, SB * n * ZPAD], CHART_DT, tag="RBb")
        ident = p_persist.tile([128, 128], F32, tag="ident")
        make_identity(nc, ident[:, :])
        startT = p_persist.tile([R, 1], F32, tag="startT")
        RB = [RBa, RBb]
        # zero-init stacks: whole-range shift DMAs read the dead inter-group
        # rows, and the simulator requires initialized reads
        nc.gpsimd.memset(L[:, :], 0.0)
        nc.gpsimd.memset(RBa[:, :], 0.0)
        nc.gpsimd.memset(RBb[:, :], 0.0)
        if XPAD > R:
            nc.gpsimd.memset(rulesYX[:, :], 0.0)

        def L5(t, w=R):
            return t.rearrange(
                "(g q) (b p y) -> g q b p y", g=G, q=32, b=SB, p=n, y=w
            )

        # ---- init: terminals into L row0 and RB(span2) row0 ----
        oh_g = d_oh.rearrange("(g b) p y -> g b p y", g=G, b=SB)
        nc.sync.dma_start(out=L5(L)[:, 0], in_=oh_g)
        for g in range(G):  # per-(g,b): keeps each DMA AP <= 3 dims
            rbv = RB[0][32 * g : 32 * g + 1].rearrange(
                "q (b p z) -> q b p z", b=SB, p=n, z=ZPAD
            )
            for b_l in range(SB):
                nc.sync.dma_start(
                    out=rbv[:, b_l, 0 : n - 1, 0:R], in_=oh_g[g, b_l, 1:n]
                )

        # ---- rules softmax (logits ~ N(0, 0.01): no max-subtract needed) ----
        SM = p_big.tile([R, R * R], F32, tag="big")
        nc.sync.dma_start(out=SM[:, :], in_=d_logits)
        nc.scalar.activation(
            out=SM[:, :], in_=SM[:, :], func=mybir.ActivationFunctionType.Exp
        )
        ssum = p_small.tile([R, 1], F32)
        nc.vector.reduce_sum(out=ssum[:, :], in_=SM[:, :], axis=mybir.AxisListType.X)
        rinv = p_small.tile([R, 1], F32)
        nc.vector.reciprocal(out=rinv[:, :], in_=ssum[:, :])
        nc.vector.tensor_scalar_mul(SM[:, :], SM[:, :], rinv[:, 0:1])
        # SM is [x, (y, z)]; rulesYX needs [z, (y, x)]: 96 PE transposes
        for y in range(R):
            trp = pp_tr.tile([R, R], F32)
            nc.tensor.transpose(
                out=trp[:, :], in_=SM[:, y * R : (y + 1) * R], identity=ident[:R, :R]
            )
            nc.vector.tensor_copy(
                out=rulesYX[:, y * XPAD : y * XPAD + R], in_=trp[:, :]
            )

        # ---- start softmax -> startT [96, 1] ----
        st = p_small.tile([1, R], F32)
        nc.sync.dma_start(out=st[:, :], in_=d_slog)
        nc.scalar.activation(
            out=st[:, :], in_=st[:, :], func=mybir.ActivationFunctionType.Exp
        )
        st_sum = p_small.tile([1, 1], F32)
        nc.vector.reduce_sum(out=st_sum[:, :], in_=st[:, :], axis=mybir.AxisListType.X)
        st_rinv = p_small.tile([1, 1], F32)
        nc.vector.reciprocal(out=st_rinv[:, :], in_=st_sum[:, :])
        nc.vector.tensor_scalar_mul(st[:, :], st[:, :], st_rinv[:, 0:1])
        nc.sync.dma_start(out=startT[:, 0:1], in_=st[0:1, :])

        # ---- span machinery ----

        def emit_pair_round(s, l0, nl, r0, r1, pairT):
            """Pair matmuls for l-indices [r0, r1) (<=5) of the window
            [l0, l0+nl), staging into pairT slots g*nl + (l - l0)."""
            P = n - s + 1
            rb = RB[s % 2]
            banks = [
                pp_pair.tile([ZPAD, 480], F32, name=f"bank{g}", tag="bank")
                for g in range(G)
            ]
            for dl in range(r1 - r0):
                ll = r0 + dl
                b_l, p = ll // P, ll % P
                off = (b_l * n + p) * ZPAD
                offL = (b_l * n + p) * R
                for g in range(G):
                    lhsT = rb[32 * g : 32 * g + s - 1, off : off + ZPAD]
                    rhs = L[32 * g : 32 * g + s - 1, offL : offL + R]
                    if PAIR_F32R:
                        lhsT = lhsT.bitcast(mybir.dt.float32r)
                        rhs = rhs.bitcast(mybir.dt.float32r)
                    nc.tensor.matmul(
                        banks[g][0:ZPAD, dl * R : (dl + 1) * R], lhsT=lhsT, rhs=rhs,
                        tile_position=(32 * g, 0),
                    )
            nr = r1 - r0
            for g in range(G):
                slot0 = g * nl + (r0 - l0)
                nc.vector.tensor_copy(
                    out=pairT[:, slot0 * R : (slot0 + nr) * R],
                    in_=banks[g][0:R, 0 : nr * R],
                )

        class ValState:
            """Tracks partially-emitted val matmuls for one window."""

            def __init__(self, s, l0, l1, pairT):
                self.s, self.l0, self.l1, self.pairT = s, l0, l1, pairT
                self.nl = l1 - l0
                self.nw = G * self.nl
                self.y = 0
                self.vps = pp_val.tile([XPAD, max(self.nw, 1)], F32)

            def emit_ys(self, count):
                nw = self.nw
                pairT_v = self.pairT.rearrange("z (it y) -> z it y", y=R)
                y1 = min(self.y + count, R)
                for y in range(self.y, y1):
                    rhs = pairT_v[:, 0:nw, y : y + 1]
                    lhsT = rulesYX[:, y * XPAD : y * XPAD + XPAD]
                    if VAL_F32R:
                        rhs = rhs.bitcast(mybir.dt.float32r)
                        lhsT = lhsT.bitcast(mybir.dt.float32r)
                    nc.tensor.matmul(
                        self.vps[0:XPAD, 0:nw], lhsT=lhsT, rhs=rhs,
                        start=(y == 0), stop=(y == R - 1),
                    )
                self.y = y1

            def finish(self):
                self.emit_ys(R - self.y)
                s, l0, l1, nl, nw = self.s, self.l0, self.l1, self.nl, self.nw
                P = n - s + 1
                if s == n:
                    # final span: dense copy; slots g*3+b_l = local sentence
                    vsb = p_valsb.tile([R, 128], F32, tag="vsb")
                    nc.vector.tensor_copy(
                        out=vsb[:, 0:nw], in_=self.vps[0:R, 0:nw]
                    )
                    ops = pp_tr.tile([1, BLOC], F32, tag="trp")
                    nc.tensor.matmul(
                        ops[0:1, 0:nw], lhsT=startT[:, 0:1], rhs=vsb[:, 0:nw]
                    )
                    osb = p_small.tile([1, BLOC], F32)
                    nc.vector.tensor_copy(out=osb[0:1, 0:nw], in_=ops[0:1, 0:nw])
                    nc.sync.dma_start(out=d_out, in_=osb[0:1, 0:nw])
                    return
                vsb = p_valsb.tile([R, 128], F32, tag="vsb")
                nc.vector.tensor_copy(out=vsb[:, 0:nw], in_=self.vps[0:R, 0:nw])
                trp = pp_tr.tile([128, R], F32)
                nc.tensor.transpose(
                    out=trp[0:nw, :], in_=vsb[:, 0:nw], identity=ident[:R, :R]
                )
                vtt = p_valt.tile([128, R], CHART_DT)
                nc.vector.tensor_copy(out=vtt[0:nw, :], in_=trp[0:nw, :])
                rbn = RB[(s + 1) % 2]

                def pview(t, part, w=R):  # one partition row view
                    return t[part : part + 1].rearrange(
                        "q (b p y) -> q b p y", b=SB, p=n, y=w
                    )

                engs = [nc.gpsimd, nc.scalar, nc.sync, nc.gpsimd]
                for b_l in range(l0 // P, (l1 - 1) // P + 1):
                    la, lb = max(l0, b_l * P), min(l1, (b_l + 1) * P)
                    pa, pb = la - b_l * P, lb - b_l * P
                    pa2 = max(pa, 1)
                    for g in range(G):
                        src = vtt[g * nl + la - l0 : g * nl + lb - l0, :]
                        engs[g].dma_start(
                            out=pview(L, 32 * g + s - 1)[:, b_l, pa:pb],
                            in_=src,
                        )
                        if pa2 < pb:
                            s2 = src[pa2 - pa :] if pa2 > pa else src
                            engs[(g + 1) % G].dma_start(
                                out=pview(rbn, 32 * g, ZPAD)[
                                    :, b_l, pa2 - 1 : pb - 1, 0:R
                                ],
                                in_=s2,
                            )

        prev: ValState | None = None
        for s in range(2, n + 1):
            P = n - s + 1
            n_l = SB * P
            if s < n:  # RB shift for next span (all partitions at once;
                # dead inter-group rows shift zeros/garbage harmlessly)
                Pn = P - 1
                src = RB[s % 2][0:127].rearrange(
                    "q (b p y) -> q b p y", b=SB, p=n
                )[:, :, 1 : Pn + 1]
                dst = RB[(s + 1) % 2][1:128].rearrange(
                    "q (b p y) -> q b p y", b=SB, p=n
                )[:, :, 0:Pn]
                nc.sync.dma_start(out=dst, in_=src)
            for l0, l1 in _windows(n_l):
                # one allocation per window: the pool's 2 slots alternate
                # across consecutive windows (also across span boundaries)
                pairT = p_big.tile([R, WCAP * R], PAIRT_DT, tag="big")
                rounds = [(r0, min(r0 + 5, l1)) for r0 in range(l0, l1, 5)]
                if SEQUENTIAL:
                    for r0, r1 in rounds:
                        emit_pair_round(s, l0, l1 - l0, r0, r1, pairT)
                    cur = ValState(s, l0, l1, pairT)
                    cur.finish()
                    continue
                for r0, r1 in rounds:
                    emit_pair_round(s, l0, l1 - l0, r0, r1, pairT)
                    if prev is not None:
                        prev.emit_ys(24)
                if prev is not None:
                    prev.finish()
                prev = ValState(s, l0, l1, pairT)
            # flush before the next span: Tile dep tracking is trace-order
            # based, so span s+1 pair matmuls must be traced after span s's
            # writebacks
            if prev is not None:
                prev.finish()
                prev = None
        if prev is not None:
            prev.finish()

    nc.compile()
    return nc


_CACHED = {}


def _get_program(n=NTOK):
    if n not in _CACHED:
        _CACHED[n] = build_program(n)
    return _CACHED[n]


def host_prep(binary_logits, start_logits, tokens, n):
    B = tokens.shape[0]
    oh = np.zeros((B, n, R), dtype=np.float32)
    bi = np.arange(B)[:, None]
    pi = np.arange(n)[None, :]
    oh[bi, pi, np.asarray(tokens).astype(np.int64)] = SCALE
    oh = np.ascontiguousarray(oh.astype(np.dtype(mybir.dt.np(CHART_DT))))
    blog = np.ascontiguousarray(
        np.asarray(binary_logits, dtype=np.float32).reshape(R, R * R)
    )
    slog = np.ascontiguousarray(
        np.asarray(start_logits, dtype=np.float32).reshape(1, R)
    )
    return blog, slog, oh


# final-span slots are g*3 + b_l = local sentence index (identity)
_SLOT_OF_BLOC = np.arange(BLOC)


TRACE = False
LAST_RESULT = None  # BassKernelResults of the most recent run (for profiling)


def kernel(binary_logits, start_logits, tokens):
    global LAST_RESULT
    tokens = np.asarray(tokens)
    n = tokens.shape[1]
    blog, slog, oh = host_prep(binary_logits, start_logits, tokens, n)
    nc = _get_program(n)
    in_maps = []
    for c in range(NCORES):
        oh_c = np.ascontiguousarray(oh[c * BLOC : (c + 1) * BLOC])
        in_maps.append({"blogits": blog, "slog": slog, "oh": oh_c})
    res = run_bass_kernel_spmd(
        nc, in_maps, core_ids=list(range(NCORES)), trace=TRACE
    )
    LAST_RESULT = res
    outs = []
    for c in range(NCORES):
        o = res.results[c]["out"].reshape(BLOC)
        outs.append(o[_SLOT_OF_BLOC])
    full = np.concatenate(outs).astype(np.float64) / (float(SCALE) ** n)
    return full.astype(np.float32)


if __name__ == "__main__":
    rng = np.random.default_rng(0)
    bl = (rng.standard_normal((R, R, R)) * 0.01).astype(np.float32)
    sl = rng.standard_normal(R).astype(np.float32)
    tk = rng.integers(0, R, (96, NTOK)).astype(np.int32)
    got = kernel(bl, sl, tk)
    print("kernel out:", got[:6])



# revision 3
# speedup vs baseline: 1.6311x; 1.6311x over previous
"""CYK/PCFG inside-algorithm kernel for Trainium2 (8 NeuronCores).

Problem: R=96 nonterminals, 96 sentences x 24 tokens.
  rules = softmax(binary_logits over (y,z)); start = softmax(start_logits)
  chart DP over span length; out[b] = start . chart[b, 0, n-1]

Sharding: data-parallel over sentences, 12 per core; rules replicated.
Rules/start softmax and the terminal one-hot run on host (f64); the device
gets pre-transposed rulesYX [z, (y,x)] and one-hot terminals.

Device layout (per core):
  - 12 sentences split into G=4 partition-groups x SB=3 sentences
    (sentence b = 3*g + b_l).
  - L stack:  L[32g + k, (b_l, p, y)]  = chart[b, p, p+k]        (left ops)
  - RB stack: RB[32g + k, (b_l, p, z)] = chart[b, p+k+1, p+s-1]  (right ops)
      Rebuilt per span by gather DMAs from L: RB_s[k, (b,p)] = L[s-2-k,
      (b, p+k+1)] for k>=1; row 0 comes from the previous span's val
      writeback (dual write). Ping-pong buffers across spans.
  - pair matmul (per item): out[z,y] = sum_k RB[k,z] * L[k,y], K=s-1<=23;
    four concurrent row-group matmuls at partition bases 0/32/64/96.
    In bf16 the lhsT reads 128 cols (FWL) while storage pitch is 96; the
    32 garbage output rows land in unused PSUM partitions.
  - val matmul: out[x, items] accumulated over y=0..95 with
    lhsT = rulesYX[:, y*XPAD:+XPAD] ([z,x]) and rhs = pairT (stride-96).
  - val results PE-transposed (in <=128-row group chunks) and
    DMA-scattered back into L and next RB's row 0.

Numerics: terminal init = SCALE(=96) so chart values ~ Catalan numbers,
keeping fp32 comfortably in range (true outputs ~1e-37). Host divides by
SCALE**n in float64 at the end.
"""

import math
import os as _os
import sys
from contextlib import ExitStack

import numpy as np

_REPO = "/opt/trn_rl_repo"
if _REPO not in sys.path:
    sys.path.insert(0, _REPO)

import concourse.bass as bass  # noqa: E402,F401
import concourse.tile as tile  # noqa: E402
from concourse import bacc, mybir  # noqa: E402
from concourse.bass_utils import run_bass_kernel_spmd  # noqa: E402
from concourse.masks import make_identity  # noqa: E402

R = 96          # nonterminals
NTOK = 24       # sentence length
NCORES = 8
BLOC = 12       # sentences per core
G = 4           # partition groups
SB = 3          # sentences per group
SCALE = 96.0
WCAP = 192      # pairT capacity (item slots)
ZPAD = 96       # RB storage pitch per (b,p) slot

F32 = mybir.dt.float32
BF16 = mybir.dt.bfloat16

# --- precision mode ----------------------------------------------------------
# "f32": full fp32 ~3e-6 rel err; "bf16": bf16 operands w/ FWL ~5e-3 rel err
MODE = _os.environ.get("KERNEL_MODE", "bf16")
if MODE == "bf16":
    CHART_DT = BF16   # L/RB stacks (pair-matmul operands)
    PAIRT_DT = BF16   # pair staging in SBUF (val-matmul rhs)
    RULES_DT = BF16   # rulesYX (val-matmul lhsT)
    LW = 128          # pair lhsT read width (128 => FWL)
    XPAD = 128        # rules slot width (128 => FWL)
else:
    CHART_DT = F32
    PAIRT_DT = F32
    RULES_DT = F32
    LW = 96
    XPAD = 96


def _windows(n_l: int) -> list[tuple[int, int]]:
    """Split n_l l-indices into 1-2 windows (bigger first for pipelining)."""
    if n_l <= 12:
        return [(0, n_l)]
    a = min(WCAP // G, math.ceil(n_l * 0.6))
    return [(0, a), (a, n_l)]


def build_program(n: int = NTOK):
    """Build the SPMD Bass program for one core (n tokens per sentence)."""
    nc = bacc.Bacc(
        "TRN2",
        target_bir_lowering=False,
        debug=False,
        enable_asserts=False,
        num_devices=NCORES,
    )

    d_rules = nc.dram_tensor(
        "rules", [R, R * XPAD], RULES_DT, kind="ExternalInput"
    ).ap()
    d_start = nc.dram_tensor("startv", [R, 1], F32, kind="ExternalInput").ap()
    d_oh = nc.dram_tensor("oh", [BLOC, n, R], CHART_DT, kind="ExternalInput").ap()
    d_out = nc.dram_tensor("out", [1, BLOC], F32, kind="ExternalOutput").ap()

    with tile.TileContext(nc) as tc, ExitStack() as ctx:
        p_persist = ctx.enter_context(tc.tile_pool(name="persist", bufs=1))
        p_big = ctx.enter_context(tc.tile_pool(name="big", bufs=2))
        p_small = ctx.enter_context(tc.tile_pool(name="small", bufs=4))
        p_valsb = ctx.enter_context(tc.tile_pool(name="valsb", bufs=2))
        p_valt = ctx.enter_context(tc.tile_pool(name="valt", bufs=4))
        pp_pair = ctx.enter_context(tc.tile_pool(name="ppair", bufs=4, space="PSUM"))
        pp_val = ctx.enter_context(tc.tile_pool(name="pval", bufs=2, space="PSUM"))
        pp_tr = ctx.enter_context(tc.tile_pool(name="ptr", bufs=2, space="PSUM"))

        # ---- persistent tiles ----
        rulesYX = p_persist.tile([R, R * XPAD], RULES_DT, tag="rules")
        L = p_persist.tile([128, SB * n * R], CHART_DT, tag="L")
        RBa = p_persist.tile([128, SB * n * ZPAD], CHART_DT, tag="RBa")
        RBb = p_persist.tile([128, SB * n * ZPAD], CHART_DT, tag="RBb")
        ident = p_persist.tile([128, 128], F32, tag="ident")
        make_identity(nc, ident[:, :])
        startT = p_persist.tile([R, 1], F32, tag="startT")
        RB = [RBa, RBb]
        # zero-init stacks: pair lhsT over-reads (LW>ZPAD) touch neighbor
        # slots, and dead rows must hold finite values
        nc.gpsimd.memset(L[:, :], 0.0)
        nc.gpsimd.memset(RBa[:, :], 0.0)
        nc.gpsimd.memset(RBb[:, :], 0.0)

        # ---- inputs ----
        nc.sync.dma_start(out=rulesYX[:, :], in_=d_rules)
        nc.scalar.dma_start(out=startT[:, :], in_=d_start)

        # 4-partition views of the stacks: [g, q, b, w] with w = n*96 cols
        def gview(t):
            return t.rearrange("(g q) (b w) -> g q b w", g=G, q=32, b=SB, w=n * R)

        Lg, RBg = gview(L), [gview(RBa), gview(RBb)]

        # terminal init: L row0 <- oh; RB[0] row0 <- oh shifted left by one
        oh_g = d_oh.rearrange("(g b) p y -> g b p y", g=G, b=SB)
        nc.sync.dma_start(out=Lg[:, 0], in_=oh_g)
        nc.scalar.dma_start(
            out=RBg[0][:, 0, :, 0 : (n - 1) * R], in_=oh_g[:, :, 1:n]
        )

        # ---- span machinery ----

        def emit_gathers(s):
            """RB rows 1..s-1 for span s+1: RB[k,(b,p)] <- L[s-1-k,(b,p+k+1)],
            P' = n-s positions. Emitted during span s; reads L rows <= s-2
            (written by span s-1's writeback)."""
            Pp = n - s
            rbn = RBg[(s + 1) % 2]
            engs = [nc.sync, nc.scalar, nc.gpsimd]
            for k in range(1, s):
                engs[k % 3].dma_start(
                    out=rbn[:, k, :, 0 : Pp * R],
                    in_=Lg[:, s - 1 - k, :, (k + 1) * R : (k + 1 + Pp) * R],
                )

        def emit_pair_round(s, l0, nl, r0, r1, pairT):
            """Pair matmuls for l-indices [r0, r1) (<=5) of the window
            [l0, l0+nl), staging into pairT slots g*nl + (l - l0)."""
            P = n - s + 1
            rb = RB[s % 2]
            banks = [
                pp_pair.tile([128, 480], F32, name=f"bank{g}", tag="bank")
                for g in range(G)
            ]
            for dl in range(r1 - r0):
                ll = r0 + dl
                b_l, p = ll // P, ll % P
                off = (b_l * n + p) * ZPAD
                offL = (b_l * n + p) * R
                for g in range(G):
                    nc.tensor.matmul(
                        banks[g][0:LW, dl * R : (dl + 1) * R],
                        lhsT=rb[32 * g : 32 * g + s - 1, off : off + LW],
                        rhs=L[32 * g : 32 * g + s - 1, offL : offL + R],
                        tile_position=(32 * g, 0),
                    )
            nr = r1 - r0
            cengs = [nc.vector, nc.scalar, nc.vector, nc.scalar]
            for g in range(G):
                slot0 = g * nl + (r0 - l0)
                ceng = cengs[g]
                if ceng is nc.scalar:
                    ceng.activation(
                        out=pairT[:, slot0 * R : (slot0 + nr) * R],
                        in_=banks[g][0:R, 0 : nr * R],
                        func=mybir.ActivationFunctionType.Copy,
                    )
                else:
                    ceng.tensor_copy(
                        out=pairT[:, slot0 * R : (slot0 + nr) * R],
                        in_=banks[g][0:R, 0 : nr * R],
                    )

        class ValState:
            """Tracks partially-emitted val matmuls for one window."""

            def __init__(self, s, l0, l1, pairT):
                self.s, self.l0, self.l1, self.pairT = s, l0, l1, pairT
                self.nl = l1 - l0
                self.nw = G * self.nl
                self.y = 0
                self.vps = pp_val.tile([XPAD, max(self.nw, 1)], F32)

            def emit_ys(self, count):
                nw = self.nw
                pairT_v = self.pairT.rearrange("z (it y) -> z it y", y=R)
                y1 = min(self.y + count, R)
                for y in range(self.y, y1):
                    nc.tensor.matmul(
                        self.vps[0:XPAD, 0:nw],
                        lhsT=rulesYX[:, y * XPAD : y * XPAD + XPAD],
                        rhs=pairT_v[:, 0:nw, y : y + 1],
                        start=(y == 0),
                        stop=(y == R - 1),
                    )
                self.y = y1

            def finish(self):
                self.emit_ys(R - self.y)
                s, l0, l1, nl, nw = self.s, self.l0, self.l1, self.nl, self.nw
                P = n - s + 1
                if s == n:
                    # final span: out[b] = start . val[:, b]
                    vsb = p_valsb.tile([R, WCAP], F32, tag="vsb")
                    nc.vector.tensor_copy(out=vsb[:, 0:nw], in_=self.vps[0:R, 0:nw])
                    ops = pp_tr.tile([1, BLOC], F32, tag="trp")
                    nc.tensor.matmul(
                        ops[0:1, 0:nw], lhsT=startT[:, 0:1], rhs=vsb[:, 0:nw]
                    )
                    osb = p_small.tile([1, BLOC], F32)
                    nc.vector.tensor_copy(out=osb[0:1, 0:nw], in_=ops[0:1, 0:nw])
                    nc.sync.dma_start(out=d_out, in_=osb[0:1, 0:nw])
                    return
                vsb = p_valsb.tile([R, WCAP], F32, tag="vsb")
                nc.vector.tensor_copy(out=vsb[:, 0:nw], in_=self.vps[0:R, 0:nw])
                rbn = RB[(s + 1) % 2]

                def pview(t, part, w=R):  # one partition row view
                    return t[part : part + 1].rearrange(
                        "q (b p y) -> q b p y", b=SB, p=n, y=w
                    )

                engs = [nc.gpsimd, nc.scalar, nc.sync, nc.gpsimd]
                cengs = [nc.vector, nc.scalar]
                gpc = max(1, 128 // nl)  # groups per transpose chunk
                ci = 0
                for c0 in range(0, G, gpc):
                    ng = min(gpc, G - c0)
                    rows = ng * nl
                    trp = pp_tr.tile([128, R], F32, tag="trp")
                    nc.tensor.transpose(
                        out=trp[0:rows, :],
                        in_=vsb[:, c0 * nl : c0 * nl + rows],
                        identity=ident[:R, :R],
                    )
                    vtt = p_valt.tile([128, R], CHART_DT)
                    ceng = cengs[ci % 2]
                    ci += 1
                    if ceng is nc.scalar:
                        ceng.activation(
                            out=vtt[0:rows, :], in_=trp[0:rows, :],
                            func=mybir.ActivationFunctionType.Copy,
                        )
                    else:
                        ceng.tensor_copy(out=vtt[0:rows, :], in_=trp[0:rows, :])
                    for b_l in range(l0 // P, (l1 - 1) // P + 1):
                        la, lb = max(l0, b_l * P), min(l1, (b_l + 1) * P)
                        pa, pb = la - b_l * P, lb - b_l * P
                        pa2 = max(pa, 1)
                        for g in range(c0, c0 + ng):
                            src = vtt[(g - c0) * nl + la - l0 : (g - c0) * nl + lb - l0, :]
                            engs[g].dma_start(
                                out=pview(L, 32 * g + s - 1)[:, b_l, pa:pb],
                                in_=src,
                            )
                            if pa2 < pb:
                                s2 = src[pa2 - pa :] if pa2 > pa else src
                                engs[(g + 1) % G].dma_start(
                                    out=pview(rbn, 32 * g, ZPAD)[
                                        :, b_l, pa2 - 1 : pb - 1, 0:R
                                    ],
                                    in_=s2,
                                )

        prev: ValState | None = None
        for s in range(2, n + 1):
            P = n - s + 1
            n_l = SB * P
            if s < n:
                emit_gathers(s)
            for l0, l1 in _windows(n_l):
                pairT = p_big.tile([R, WCAP * R], PAIRT_DT, tag="big")
                rounds = [(r0, min(r0 + 5, l1)) for r0 in range(l0, l1, 5)]
                ys_per = -(-R // len(rounds))
                for r0, r1 in rounds:
                    emit_pair_round(s, l0, l1 - l0, r0, r1, pairT)
                    if prev is not None:
                        prev.emit_ys(ys_per)
                if prev is not None:
                    prev.finish()
                prev = ValState(s, l0, l1, pairT)
            # flush before the next span: Tile dep tracking is trace-order
            # based, so span s+1 pair matmuls must be traced after span s's
            # writebacks
            if prev is not None:
                prev.finish()
                prev = None
        if prev is not None:
            prev.finish()

    nc.compile()
    return nc


_CACHED = {}


def _get_program(n=NTOK):
    if n not in _CACHED:
        _CACHED[n] = build_program(n)
    return _CACHED[n]


def host_prep(binary_logits, start_logits, tokens, n):
    B = tokens.shape[0]
    oh = np.zeros((B, n, R), dtype=np.float32)
    bi = np.arange(B)[:, None]
    pi = np.arange(n)[None, :]
    oh[bi, pi, np.asarray(tokens).astype(np.int64)] = SCALE
    oh = np.ascontiguousarray(oh.astype(np.dtype(mybir.dt.np(CHART_DT))))
    # rules softmax in f64, laid out as rulesYX[z, (y, x)] with x padded
    bl = np.asarray(binary_logits, dtype=np.float64).reshape(R, R * R)
    e = np.exp(bl - bl.max(axis=1, keepdims=True))
    rules = (e / e.sum(axis=1, keepdims=True)).reshape(R, R, R)  # [x,y,z]
    ryx = np.zeros((R, R, XPAD), dtype=np.float64)  # [z, y, x]
    ryx[:, :, 0:R] = rules.transpose(2, 1, 0)
    ryx = np.ascontiguousarray(
        ryx.reshape(R, R * XPAD).astype(np.dtype(mybir.dt.np(RULES_DT)))
    )
    sl = np.asarray(start_logits, dtype=np.float64)
    es = np.exp(sl - sl.max())
    start = (es / es.sum()).reshape(R, 1).astype(np.float32)
    return ryx, np.ascontiguousarray(start), oh


TRACE = False
LAST_RESULT = None  # BassKernelResults of the most recent run (for profiling)


def kernel(binary_logits, start_logits, tokens):
    global LAST_RESULT
    tokens = np.asarray(tokens)
    n = tokens.shape[1]
    ryx, start, oh = host_prep(binary_logits, start_logits, tokens, n)
    nc = _get_program(n)
    in_maps = []
    for c in range(NCORES):
        oh_c = np.ascontiguousarray(oh[c * BLOC : (c + 1) * BLOC])
        in_maps.append({"rules": ryx, "startv": start, "oh": oh_c})
    res = run_bass_kernel_spmd(
        nc, in_maps, core_ids=list(range(NCORES)), trace=TRACE
    )
    LAST_RESULT = res
    outs = []
    for c in range(NCORES):
        o = res.results[c]["out"].reshape(BLOC)
        outs.append(o)
    full = np.concatenate(outs).astype(np.float64) / (float(SCALE) ** n)
    return full.astype(np.float32)


if __name__ == "__main__":
    rng = np.random.default_rng(0)
    bl = (rng.standard_normal((R, R, R)) * 0.01).astype(np.float32)
    sl = rng.standard_normal(R).astype(np.float32)
    tk = rng.integers(0, R, (96, NTOK)).astype(np.int32)
    got = kernel(bl, sl, tk)
    print("kernel out:", got[:6])
